# revision 4
# baseline (speedup 1.0000x reference)
"""Trainium2 distributed kernel for CrossRNN (grid of 2-layer ReLU RNNs +
row/col message passing + linear head), 8 NeuronCores SPMD.

Math (per grid cell): 2-layer Elman RNN (relu) over S=32 embedded tokens,
last hidden h of the top layer, then with u = h.w1, s = h.w2:
  out[b,r,c] = u - 2*s + sum_c' s[b,r,c'] + sum_r' s[b,r',c] + pred_b

Sharding: core k owns sample b=k//2, rows [32*(k%2), 32*(k%2)+32) => 2048
independent sequences/core. Row sums are local; the column-sum partials are
combined on the HOST during unsharding (64 floats per core).

v3 design (vs the bf16 v2 at ~107us, PE-bound at 84us):
 - All recurrent matmuls run in fp8(e4m3) DoubleRow perf mode: 0.5
   PE-cycles/output-column with TWO K=128 contraction tiles per
   instruction (4x the bf16 MAC rate).
 - Layer 1 folds the per-step input INTO the matmul with an identity
   K-tile:  p1 = [W_hh0 ; I] @ [h1_{t-1} ; g_t]  (one DoubleRow pass).
   relu1 is then a single-operand max(p1, 0) -> runs on ANY engine
   (split gpsimd / DVE), not just the 2-input custom-DVE op of v2.
 - Layer 2 uses two-term residual fp8 weights (W ~= hi + lo, quantized
   at 64x scale) to kill fp8 weight-quantization error:
   p2 = [W_hh1_hi ; W_ih1_hi]@[h2,h1] + [W_hh1_lo ; W_ih1_lo]@[h2,h1].
   relu2 = max(p2 * 2^-6 + s_h*b1, 0): ScalarE activation(scale, bias)
   plus a custom DVE op relu(Src0*C2 + C0) for a column slice, so the
   elementwise work is spread over Act+DVE+Pool ~evenly -- the
   PSUM-evacuation rate (1 elem/cycle/partition/engine) is the wall
   once PE runs at fp8 DoubleRow rate.
 - Activations h1,h2 are stored as fp8 at scale s_h=48 (h absmax ~0.23);
   numpy simulation of this exact scheme gives rel_err 4.6e-3 (vs
   4.4e-3 for the all-bf16 v2; tolerance 2e-2).
 - Per step there is ONE shared SBUF tile Z_t [128, 3, 2048] fp8 holding
   slot0=h2_{t-1}, slot1=h1_t, slot2=g_{t+1}; L2 reads slots (0,1), L1
   reads slots (1,2), so both DoubleRow rhs APs are contiguous 2-slot
   views.  Writers (relu2, relu1, g-DMA) hit disjoint slots.
 - g table is fp8 (half the v2 DMA traffic): embedding gather on HOST
   against P = fp8(s_h*(embed @ W_ih0.T + b0)), streamed per-step.
 - Last step writes h2 as bf16 and the head (u,s) matmul runs bf16.
"""

import numpy as np
import ml_dtypes

B, R, C, S = 4, 64, 64, 32
V, E, H, L = 30000, 128, 128, 2
N_CORES = 8
NPC = (B * R * C) // N_CORES  # 2048 sequences per core
ROWS_PC = 32                  # rows per core
SW = NPC // 2                 # stream width (1024) = one PSUM tile
MMW = 256                     # DoubleRow moving chunk (rhs free = 2*256)
S_H = 48.0                    # fp8 activation scale
WSC = 64.0                    # layer-2 residual weight scale (2^6)
REL2DVE = 320                 # cols of relu2(X) handled by DVE custom op
RY = 896                      # cols of relu1(Y) handled by DVE (rest Act)

_cache = {}

RELU2_NAME = "RELU_SB_XRNN2"


def _register_relu2():
    """Register out = relu(Src0*imm2 + s0) custom DVE op (idempotent)."""
    from concourse import dve_ops as DO
    from concourse.dve_spec import Spec, Src0, C0, C2, relu, lower as dve_lower
    from concourse.dve_uop import DveOpSpec
    from concourse.dve_table_gen import dve_ver_for

    if RELU2_NAME in DO._SUB_OPCODE_FOR_NAME:
        return next(op for op in DO.OPS if op.name == RELU2_NAME)

    spec = Spec(
        body=relu(Src0 * C2 + C0),
        reference=lambda in0, in1, s0, s1, imm2: np.maximum(
            in0.astype(np.float32) * imm2 + s0, 0.0
        ),
    )
    opcode = DO._CUSTOM_DVE_ROW_BASE + len(DO.OPS)
    assert opcode < 0x20
    DO._SUB_OPCODE_FOR_NAME[RELU2_NAME] = opcode
    ver = dve_ver_for("TRN2")
    sha = DveOpSpec(
        name=RELU2_NAME, opcode=opcode, uops=dve_lower(spec, ver=ver), rd1_en=False
    ).sha(ver)
    op = DO.DveOp(RELU2_NAME, spec, subdim=False, uops_sha={ver: sha})
    DO.OPS.append(op)
    DO.CUSTOM_DVE_SPECS[RELU2_NAME] = spec
    return op


def _build():
    if "nc" in _cache:
        return _cache["nc"]

    import concourse.mybir as mybir
    import concourse.tile as tile
    from concourse import bacc
    from concourse.bass import ds

    f32 = mybir.dt.float32
    bf16 = mybir.dt.bfloat16
    f8 = mybir.dt.float8e4
    DR = mybir.MatmulPerfMode.DoubleRow

    relu2_op = _register_relu2()

    nc = bacc.Bacc("TRN2", target_bir_lowering=False, debug=False,
                   num_devices=N_CORES)

    g_d = nc.dram_tensor("g", [128, S * NPC], f8, kind="ExternalInput")
    # lhsT pairs [k, pair, slot, m]: pair0=[W_hh0.T, I], pair1=[hi pair],
    # pair2=[lo pair] (L2 pairs are [W_hh1.T, W_ih1.T] slot order)
    wts_d = nc.dram_tensor("wts", [128, 3 * 2 * H], f8, kind="ExternalInput")
    # biases: col 0 = s_h*(b_ih1+b_hh1) ; col 1 = pred_b bcast
    biases_d = nc.dram_tensor("biases", [128, 2], f32, kind="ExternalInput")
    pw_d = nc.dram_tensor("pw", [128, 2], bf16, kind="ExternalInput")
    # out rows 0..31 = local acc (u - 2s + rowS + pred_b); row 32 = this
    # core's column-sum partial (cross-core combine happens on the host).
    out_d = nc.dram_tensor("out", [ROWS_PC + 1, C], f32, kind="ExternalOutput")

    with tile.TileContext(nc) as tc:
        with (
            tc.tile_pool(name="const", bufs=1) as constp,
            tc.tile_pool(name="zpool", bufs=6) as zpool,
            tc.tile_pool(name="tailp", bufs=1) as tailp,
        ):
            wts_sb = constp.tile([128, 3, 2, H], f8)
            biases_sb = constp.tile([128, 2], f32)
            pw_sb = constp.tile([128, 2], bf16)

            # consts ride the scalar HWDGE ring so the g stream (on sync)
            # starts immediately
            nc.scalar.dma_start(
                wts_sb[:, :, :, :],
                wts_d.ap().rearrange("k (p two m) -> k p two m", p=3, two=2))
            nc.scalar.dma_start(biases_sb[:], biases_d.ap())
            nc.scalar.dma_start(pw_sb[:], pw_d.ap())

            # Z_t slots: 0 = h2_{t-1}, 1 = h1_t, 2 = g_{t+1}
            Z = {}
            Z[-1] = zpool.tile([128, 3, NPC], f8, tag="z", name="z_m1")
            nc.sync.dma_start(Z[-1][:, 2, :], g_d.ap()[:, ds(0, NPC)])
            # zero initial h1_{-1} (slot1 of Z[-1]); h2_{-1} is slot0 of Z[0]
            nc.gpsimd.memset(Z[-1][:, 1, :], 0.0)

            h2_last = tailp.tile([128, NPC], bf16)

            with (
                tc.tile_pool(name="p1x", bufs=1, space="PSUM") as p1xp,
                tc.tile_pool(name="p1y", bufs=1, space="PSUM") as p1yp,
                tc.tile_pool(name="p2x", bufs=1, space="PSUM") as p2xp,
                tc.tile_pool(name="p2y", bufs=1, space="PSUM") as p2yp,
            ):
                p1 = [p1xp.tile([128, SW], f32, name="p1x"),
                      p1yp.tile([128, SW], f32, name="p1y")]
                p2 = [p2xp.tile([128, SW], f32, name="p2x"),
                      p2yp.tile([128, SW], f32, name="p2y")]

                for t in range(S + 1):
                    s = t - 1  # layer-2 step handled this tick
                    if t <= S - 1:
                        Z[t] = zpool.tile([128, 3, NPC], f8, tag="z",
                                          name=f"z{t}")
                        if t == 0:
                            nc.vector.memset(Z[0][:, 0, :], 0.0)
                        if t <= S - 2:
                            nc.sync.dma_start(
                                Z[t][:, 2, :],
                                g_d.ap()[:, ds((t + 1) * NPC, NPC)])

                    # ---- layer 2 of step s: p2 = hi@[h2,h1] + lo@[h2,h1]
                    if s >= 0:
                        for st in range(2):
                            off = st * SW
                            for c in range(SW // MMW):
                                rhs = Z[s][:, 0:2, ds(off + c * MMW, MMW)]
                                nc.tensor.matmul(p2[st][:, ds(c * MMW, MMW)],
                                                 wts_sb[:, 1, :, :], rhs,
                                                 start=True, stop=False,
                                                 perf_mode=DR)
                                nc.tensor.matmul(p2[st][:, ds(c * MMW, MMW)],
                                                 wts_sb[:, 2, :, :], rhs,
                                                 start=False, stop=True,
                                                 perf_mode=DR)
                        # relu2(s) = max(p2*2^-6 + b1', 0) -> slot0 of Z[s+1]
                        # (bf16 h2_last on the final step); X gets a DVE
                        # slice so Act isn't the elementwise bottleneck
                        if s == S - 1:
                            dx = h2_last[:, 0:SW]
                            dy = h2_last[:, SW:NPC]
                        else:
                            dx = Z[s + 1][:, 0, 0:SW]
                            dy = Z[s + 1][:, 0, SW:NPC]
                        nc.vector._custom_dve(
                            relu2_op,
                            out=dx[:, 0:REL2DVE],
                            in0=p2[0][:, 0:REL2DVE],
                            s0=biases_sb[:, 0:1],
                            imm2=1.0 / WSC)
                        nc.scalar.activation(
                            dx[:, REL2DVE:SW], p2[0][:, REL2DVE:SW],
                            mybir.ActivationFunctionType.Relu,
                            bias=biases_sb[:, 0:1], scale=1.0 / WSC)
                        nc.scalar.activation(
                            dy, p2[1][:],
                            mybir.ActivationFunctionType.Relu,
                            bias=biases_sb[:, 0:1], scale=1.0 / WSC)

                    # ---- layer 1 of step t: p1 = [W_hh0 ; I] @ [h1 ; g]
                    if t <= S - 1:
                        for st in range(2):
                            off = st * SW
                            for c in range(SW // MMW):
                                rhs = Z[t - 1][:, 1:3, ds(off + c * MMW, MMW)]
                                nc.tensor.matmul(p1[st][:, ds(c * MMW, MMW)],
                                                 wts_sb[:, 0, :, :], rhs,
                                                 start=True, stop=True,
                                                 perf_mode=DR)
                        # relu1(t) = max(p1, 0) -> slot1 of Z[t].
                        # GPSIMD cannot read PSUM on TRN2, so the
                        # elementwise work is split DVE / Act only.
                        nc.vector.tensor_scalar_max(
                            Z[t][:, 1, 0:SW], p1[0][:], 0.0)
                        nc.vector.tensor_scalar_max(
                            Z[t][:, 1, SW:SW + RY], p1[1][:, 0:RY], 0.0)
                        nc.scalar.activation(
                            Z[t][:, 1, SW + RY:NPC], p1[1][:, RY:SW],
                            mybir.ActivationFunctionType.Relu)

            # ---- head: u = h.w1, s = h.w2 (psum [2, 512] in 4 chunks) ----
            CW = 512
            us_sb = tailp.tile([2, NPC], f32)
            with tc.tile_pool(name="usp", bufs=2, space="PSUM") as usp:
                for c in range(NPC // CW):
                    pus = usp.tile([2, CW], f32, tag="us")
                    nc.tensor.matmul(pus[:], pw_sb[:],
                                     h2_last[:, ds(c * CW, CW)],
                                     start=True, stop=True)
                    nc.vector.tensor_copy(us_sb[:, ds(c * CW, CW)], pus[:])

            # spread s to [rows, cols]; col-sum partial via ones-matmul
            s_rc = tailp.tile([ROWS_PC, C], f32)
            nc.sync.dma_start(s_rc[:], us_sb[1:2, :].rearrange("p (r c) -> p r c", r=ROWS_PC))
            u_rc = tailp.tile([ROWS_PC, C], f32)
            nc.sync.dma_start(u_rc[:], us_sb[0:1, :].rearrange("p (r c) -> p r c", r=ROWS_PC))
            ones_sb = tailp.tile([ROWS_PC, 1], f32)
            nc.vector.memset(ones_sb[:], 1.0)
            acc = tailp.tile([ROWS_PC + 1, C], f32)
            with tc.tile_pool(name="cspp", bufs=1, space="PSUM") as cspp:
                csp_ps = cspp.tile([1, C], f32)
                nc.tensor.matmul(csp_ps[:], ones_sb[:], s_rc[:], start=True, stop=True)
                nc.vector.tensor_copy(acc[ROWS_PC:ROWS_PC + 1, :], csp_ps[:])

            # local part: u - 2s + rowS (+pred_b); colsum partial rides as
            # row 32 of the output and is combined across cores on the host
            rowS = tailp.tile([ROWS_PC, 1], f32)
            nc.vector.tensor_reduce(rowS[:], s_rc[:], axis=mybir.AxisListType.X,
                                    op=mybir.AluOpType.add)
            nc.vector.tensor_add(rowS[:], rowS[:], biases_sb[0:ROWS_PC, 1:2])
            nc.vector.scalar_tensor_tensor(acc[0:ROWS_PC, :], s_rc[:], -2.0, u_rc[:],
                                           mybir.AluOpType.mult, mybir.AluOpType.add)
            nc.vector.tensor_scalar(acc[0:ROWS_PC, :], acc[0:ROWS_PC, :], rowS[:],
                                    None, mybir.AluOpType.add)
            nc.sync.dma_start(out_d.ap(), acc[:])

    nc.compile()
    _cache["nc"] = nc
    return nc


def _prep_in_maps(inputs):
    x = np.asarray(inputs["x"])
    embed = np.asarray(inputs["embed"], dtype=np.float32)
    W_ih = np.asarray(inputs["W_ih"], dtype=np.float32)
    W_hh = np.asarray(inputs["W_hh"], dtype=np.float32)
    b_ih = np.asarray(inputs["b_ih"], dtype=np.float32)
    b_hh = np.asarray(inputs["b_hh"], dtype=np.float32)
    pred_W = np.asarray(inputs["pred_W"], dtype=np.float32)
    pred_b = np.asarray(inputs["pred_b"], dtype=np.float32)
    bf16 = ml_dtypes.bfloat16
    f8 = ml_dtypes.float8_e4m3

    # fold layer-1 input projection + bias + activation scale into the
    # fp8 gather table
    b0 = b_ih[0] + b_hh[0]
    b1 = (b_ih[1] + b_hh[1]) * S_H
    P_f8 = ((embed @ W_ih[0].T + b0) * S_H).astype(f8)  # [V, 128]

    # host gather: per core [128(E), S, 2048] then flatten cols
    gath = P_f8[x]  # [4, 64, 64, 32, 128]
    gath = gath.reshape(B, 2, ROWS_PC, C, S, E)

    # lhsT pairs [k, pair, slot, m]; pair0 = [W_hh0.T, I],
    # pair1/2 = [W_hh1.T, W_ih1.T] hi/lo residual halves at 64x scale
    q = lambda a: a.astype(f8).astype(np.float32)
    Wi1_hi = q(WSC * W_ih[1]); Wi1_lo = (WSC * W_ih[1] - Wi1_hi).astype(f8)
    Wh1_hi = q(WSC * W_hh[1]); Wh1_lo = (WSC * W_hh[1] - Wh1_hi).astype(f8)
    eye = np.eye(H, dtype=np.float32)
    pairs = np.stack([
        np.stack([W_hh[0].T, eye], axis=1),
        np.stack([Wh1_hi.T, Wi1_hi.T], axis=1),
        np.stack([Wh1_lo.astype(np.float32).T, Wi1_lo.astype(np.float32).T], axis=1),
    ], axis=1)  # [k, 3, 2, m]
    wts = np.ascontiguousarray(pairs.reshape(128, 3 * 2 * H)).astype(f8)
    biases = np.stack([b1, np.full(H, pred_b[0], np.float32)], axis=1).astype(np.float32)
    pw = np.ascontiguousarray((pred_W[0] / S_H).reshape(2, H).T.astype(bf16))

    in_maps = []
    for k in range(N_CORES):
        b, rh = k // 2, k % 2
        # [32, 64, 32, 128] -> [128(E), 32(S), 2048(n=r*64+c)]
        g = gath[b, rh].reshape(NPC, S, E).transpose(2, 1, 0)
        g = np.ascontiguousarray(g).reshape(128, S * NPC)
        in_maps.append({
            "g": g, "wts": wts, "biases": biases, "pw": pw,
        })
    return in_maps


def run(inputs, trace=False):
    from concourse import bass_utils
    nc = _build()
    in_maps = _prep_in_maps(inputs)
    res = bass_utils.run_bass_kernel_spmd(
        nc, in_maps, core_ids=list(range(N_CORES)), trace=trace,
    )
    out = np.empty((B, R, C), np.float32)
    colS = np.zeros((B, C), np.float32)
    for k in range(N_CORES):
        b, r0 = k // 2, ROWS_PC * (k % 2)
        block = res.results[k]["out"]
        out[b, r0:r0 + ROWS_PC, :] = block[:ROWS_PC]
        colS[b] += block[ROWS_PC]
    out += colS[:, None, :]
    return out, res


def kernel(**inputs):
    out, _ = run(inputs, trace=False)
    return out
